# revision 17
# baseline (speedup 1.0000x reference)
"""Luong 'general' attention kernel for Trainium2 (Bass/Tile), 8-core SPMD.

Math (per batch b):
    v_b        = Wa @ dec_ht[b]                      # (H,)  -- tiny, replaces the
                                                     # huge (T,H)@(H,U) matmul
    raw[t]     = enc_hs[b,t,:] . v_b                 # (T,)
    score[t]   = mask[b,t] ? raw[t] : -1e9           # exact: masked enc gives raw*0
    attn       = softmax(score)                      # masked lanes underflow to 0
    context[b] = sum_t attn[t] * enc_hs[b,t,:]       # masked lanes contribute 0*enc

Sharding: data-parallel over batch B=32 across 8 cores (4 batches/core),
Wa replicated.  Single pass over enc_hs per core (33.5 MB streamed).

Engine plan per core:
  - DVE    : enc*v products for scores (fp32, full precision)
  - ACT    : free-dim reduce of the products (activation Copy + accum), exp
  - PE     : Wa transpose + V matmuls (fp32), context matmuls in f32r
             (single-pass fp32 matmul: 1 cycle/row vs 4 for exact fp32,
             ~2e-4 rel err -- scores still read the same tiles as exact
             f32 via bitcast)
  - DMA    : streams enc (32 MB) + Wa (4 MB) -> the ~100 us roofline
"""

import os
import sys
from contextlib import ExitStack

for _p in ("/root/.axon_site", "/root/.axon_site/_ro/trn_rl_repo",
           "/root/.axon_site/_ro/pypackages", "/opt/trn_rl_repo"):
    if os.path.isdir(_p) and _p not in sys.path:
        sys.path.append(_p)

import numpy as np

import concourse.bass as bass
import concourse.tile as tile
from concourse import bacc, masks, mybir

B, T, H, U = 32, 2048, 1024, 1024
N_CORES = 8
B_LOC = B // N_CORES          # 4 batches per core
TCH = T // 128                # 16 t-chunks of 128 per batch
NEG_BIG = -1.0e9
F32 = mybir.dt.float32
BF16 = mybir.dt.bfloat16
F32R = mybir.dt.float32r


def emit_kernel(tc, enc, dec, mask, wa, out):
    """Emit the per-core program.  enc:[B_LOC,T,H] dec:[B_LOC,H] mask:[B_LOC,T]u8
    wa:[H,U] out:[B_LOC,H], all DRAM APs."""
    nc = tc.nc
    with ExitStack() as ctx:
        const_pool = ctx.enter_context(tc.tile_pool(name="const", bufs=1))
        ident = const_pool.tile([128, 128], F32, tag="ident")
        masks.make_identity(nc, ident[:])
        ones_col = const_pool.tile([128, 1], F32, tag="ones_col")
        nc.vector.memset(ones_col[:], 1.0)
        neg_row = const_pool.tile([1, 128], F32, tag="neg_row")
        nc.vector.memset(neg_row[:], -1.0)
        neg_big = const_pool.tile([128, TCH], F32, tag="neg_big")
        nc.vector.memset(neg_big[:], NEG_BIG)

        # enc tiles are declared float32r so the context matmul takes the
        # fast single-pass fp32 path; the score ops bitcast them to f32.
        # Two pools: the second opens after the V-phase transients release,
        # reusing that SBUF for deeper DMA prefetch.
        enc_pool = ctx.enter_context(tc.tile_pool(name="enc", bufs=20))
        vrep_pool = ctx.enter_context(tc.tile_pool(name="vrep", bufs=1))
        scr_pool = ctx.enter_context(tc.tile_pool(name="scr", bufs=3))
        small_pool = ctx.enter_context(tc.tile_pool(name="small", bufs=2))

        # ---------- Phase V: v_rep[b][p, h] = (Wa @ dec[b])[h] for all p ----------
        vreps = []
        with ExitStack() as vctx:
            wa_pool = vctx.enter_context(tc.tile_pool(name="wa", bufs=1))
            waT_pool = vctx.enter_context(tc.tile_pool(name="waT", bufs=2))
            psum_tr = vctx.enter_context(
                tc.tile_pool(name="psum_tr", bufs=4, space="PSUM"))
            psum_v = vctx.enter_context(
                tc.tile_pool(name="psum_v", bufs=1, space="PSUM"))
            vsb_pool = vctx.enter_context(tc.tile_pool(name="vsb", bufs=1))

            # dec transposed: dT[p, c, b] = dec[b, c*128+p]
            dT = vsb_pool.tile([128, 8, B_LOC], F32R, tag="dT")
            for c in range(8):
                nc.sync.dma_start(
                    dT[:, c, :],
                    dec[:, c * 128:(c + 1) * 128].rearrange("b p -> p b"))

            wa_tiles = []
            for i in range(8):  # h-chunk
                wt = wa_pool.tile([128, U], F32, name=f"wa_{i}", tag=f"wa_{i}")
                for hf in range(4):  # split across four DMA queues
                    nc.sync.dma_start(
                        wt[hf * 32:(hf + 1) * 32, :],
                        wa[i * 128 + hf * 32:i * 128 + (hf + 1) * 32, :])
                wa_tiles.append(wt)

            # vT[b, h] accumulated over u-chunks j
            vT_ps = psum_v.tile([B_LOC, H], F32, tag="vT_ps")
            for j in range(8):  # u-chunk
                waT_sb = waT_pool.tile([128, H], F32R, name="waT_sb",
                                       tag="waT_sb", bufs=2)
                for hh in range(2):
                    tr_ps = psum_tr.tile([128, 512], F32, name="tr_ps",
                                         tag="tr_ps", bufs=4)
                    for k in range(4):
                        i = hh * 4 + k  # h-chunk
                        nc.tensor.transpose(
                            tr_ps[:, k * 128:(k + 1) * 128],
                            wa_tiles[i][:, j * 128:(j + 1) * 128],
                            ident[:])
                    # evacuate on DVE / ACT alternately (both idle here)
                    eng = nc.vector if hh == 0 else nc.scalar
                    if eng is nc.vector:
                        nc.vector.tensor_copy(
                            waT_sb[:, hh * 512:(hh + 1) * 512], tr_ps[:])
                    else:
                        nc.scalar.copy(
                            waT_sb[:, hh * 512:(hh + 1) * 512], tr_ps[:])
                for hh in range(2):
                    nc.tensor.matmul(
                        vT_ps[:, hh * 512:(hh + 1) * 512],
                        dT[:, j, :],
                        waT_sb[:, hh * 512:(hh + 1) * 512],
                        start=(j == 0), stop=(j == 7))

            vT_sb = vsb_pool.tile([B_LOC, H], F32, tag="vT_sb")
            nc.scalar.copy(vT_sb[:], vT_ps[:])

            for b in range(B_LOC):
                # row b -> partition 0 (engines can only address partition
                # bases 0/32/64), then broadcast to all 128 partitions
                vb_sb = vsb_pool.tile([1, H], F32, name=f"vb_{b}",
                                      tag=f"vb_{b}")
                nc.sync.dma_start(vb_sb[:], vT_sb[b:b + 1, :])
                vrep = vrep_pool.tile([128, H], F32, name=f"vrep_{b}",
                                      tag=f"vrep_{b}")
                nc.gpsimd.partition_broadcast(vrep[:], vb_sb[:])
                vreps.append(vrep)

        # PSUM pools for the steady state, opened after the V-phase PSUM
        # pools released (only 8 banks exist)
        psum_small = ctx.enter_context(
            tc.tile_pool(name="psum_small", bufs=2, space="PSUM"))
        psum_ctx = ctx.enter_context(
            tc.tile_pool(name="psum_ctx", bufs=2, space="PSUM"))
        enc_pool2 = ctx.enter_context(tc.tile_pool(name="enc2", bufs=21))

        # ---------- Main per-batch pipeline ----------
        for b in range(B_LOC):
            enc_tiles = []
            for i in range(TCH):
                # round-robin tiles across the two pools
                if (b * TCH + i) % 41 < 20:
                    et = enc_pool.tile([128, H], F32R, name="enc_t",
                                       tag="enc_t", bufs=20)
                else:
                    et = enc_pool2.tile([128, H], F32R, name="enc_t2",
                                        tag="enc_t2", bufs=21)
                nc.sync.dma_start(et[:], enc[b, i * 128:(i + 1) * 128, :])
                enc_tiles.append(et)

            # mask (uint8 0/1), laid out [p, i] ~ t = i*128+p
            mk_u8 = small_pool.tile([128, TCH], mybir.dt.uint8, name="mk_u8",
                                    tag="mk_u8", bufs=2)
            nc.sync.dma_start(mk_u8[:],
                               mask[b].rearrange("(i p) -> p i", p=128))

            # scores: DVE multiplies enc*v, ACT reduces over the free dim via
            # activation(Copy) with accum_out (output goes to a broadcast sink)
            sm = small_pool.tile([128, TCH], F32, name="sm", tag="sm", bufs=2)
            for i in range(TCH):
                scr = scr_pool.tile([128, H], F32, name="scr", tag="scr", bufs=3)
                nc.vector.tensor_mul(scr[:], enc_tiles[i][:].bitcast(F32),
                                     vreps[b][:])
                sink = scr_pool.tile([128, 1], F32, name="sink", tag="sink",
                                     bufs=3)
                nc.scalar.activation(sink.broadcast_to(scr.shape), scr[:],
                                     mybir.ActivationFunctionType.Copy,
                                     bias=0.0, scale=1.0,
                                     accum_out=sm[:, i:i + 1])

            # masked score: where(mask, sm, -1e9)
            smm = small_pool.tile([128, TCH], F32, name="smm", tag="smm", bufs=2)
            nc.vector.tensor_copy(smm[:], neg_big[:])
            nc.vector.copy_predicated(smm[:], mk_u8[:], sm[:])

            # global max over [128, TCH]
            colmax = small_pool.tile([128, 1], F32, name="colmax", tag="colmax",
                                     bufs=2)
            nc.vector.reduce_max(colmax[:], smm[:], axis=mybir.AxisListType.X)
            sp = psum_small.tile([128, 160], F32, name="sp", tag="sp", bufs=2)
            nc.tensor.transpose(sp[:1, 0:128], colmax[:], ident[:])
            gmax = small_pool.tile([1, 1], F32, name="gmax", tag="gmax", bufs=2)
            nc.vector.reduce_max(gmax[:], sp[:1, 0:128],
                                 axis=mybir.AxisListType.X)
            # negmax[p] = -gmax, replicated across partitions
            nc.tensor.matmul(sp[:, 128:129], neg_row[:], gmax[:])
            negmax = small_pool.tile([128, 1], F32, name="negmax", tag="negmax",
                                     bufs=2)
            nc.vector.tensor_copy(negmax[:], sp[:, 128:129])

            # p = exp(score - max) in f32r (feeds the f32r context matmul),
            # with the exact fp32 row-sum from the ACT accumulator
            emat = small_pool.tile([128, TCH], F32R, name="emat", tag="emat",
                                   bufs=2)
            rowsum = small_pool.tile([128, 1], F32, name="rowsum", tag="rowsum",
                                     bufs=2)
            nc.scalar.activation(emat[:], smm[:],
                                 mybir.ActivationFunctionType.Exp,
                                 bias=negmax[:], scale=1.0,
                                 accum_out=rowsum[:])

            # denom = sum_p rowsum[p]; rden = 1/denom
            nc.tensor.matmul(sp[:1, 129:130], rowsum[:], ones_col[:])
            rden = small_pool.tile([1, 1], F32, name="rden", tag="rden", bufs=2)
            nc.vector.reciprocal(rden[:], sp[:1, 129:130])

            # context[h] = sum_t p[t] * enc[t, h]  (f32r single-pass matmul)
            cps = psum_ctx.tile([1, H], F32, name="cps", tag="cps", bufs=2)
            for i in range(TCH):
                for hh in range(2):
                    nc.tensor.matmul(
                        cps[:, hh * 512:(hh + 1) * 512],
                        emat[:, i:i + 1],
                        enc_tiles[i][:, hh * 512:(hh + 1) * 512],
                        start=(i == 0), stop=(i == TCH - 1))

            ctx_sb = small_pool.tile([1, H], F32, name="ctx_sb", tag="ctx_sb",
                                     bufs=2)
            nc.vector.tensor_scalar_mul(ctx_sb[:], cps[:], rden[:])
            nc.gpsimd.dma_start(out[b:b + 1, :], ctx_sb[:])


def build_nc():
    """Build and compile the per-core Bass program."""
    nc = bacc.Bacc("TRN2", target_bir_lowering=False, debug=False,
                   enable_asserts=False, num_devices=N_CORES)
    enc_d = nc.dram_tensor("enc_hs", [B_LOC, T, H], F32R,
                           kind="ExternalInput")
    dec_d = nc.dram_tensor("dec_ht", [B_LOC, H], F32R, kind="ExternalInput")
    mask_d = nc.dram_tensor("mask", [B_LOC, T], mybir.dt.uint8,
                            kind="ExternalInput")
    wa_d = nc.dram_tensor("Wa", [H, U], F32, kind="ExternalInput")
    out_d = nc.dram_tensor("context", [B_LOC, H], F32, kind="ExternalOutput")

    with tile.TileContext(nc) as tc:
        emit_kernel(tc, enc_d.ap(), dec_d.ap(), mask_d.ap(), wa_d.ap(),
                    out_d.ap())
    nc.compile()
    return nc


def make_in_maps(enc_hs, dec_ht, mask, Wa):
    """Shard full inputs into per-core input maps (data-parallel over batch)."""
    enc_hs = np.ascontiguousarray(enc_hs, dtype=np.float32)
    dec_ht = np.ascontiguousarray(dec_ht, dtype=np.float32)
    mask_u8 = np.ascontiguousarray(mask).astype(np.uint8)
    Wa = np.ascontiguousarray(Wa, dtype=np.float32)
    in_maps = []
    for c in range(N_CORES):
        sl = slice(c * B_LOC, (c + 1) * B_LOC)
        in_maps.append({
            "enc_hs": enc_hs[sl],
            "dec_ht": dec_ht[sl],
            "mask": mask_u8[sl],
            "Wa": Wa,
        })
    return in_maps


_NC_CACHE = None


def get_nc():
    global _NC_CACHE
    if _NC_CACHE is None:
        _NC_CACHE = build_nc()
    return _NC_CACHE


def run_on_hw(enc_hs, dec_ht, mask, Wa, trace=False, **trace_kwargs):
    from concourse.bass_utils import run_bass_kernel_spmd
    nc = get_nc()
    in_maps = make_in_maps(enc_hs, dec_ht, mask, Wa)
    res = run_bass_kernel_spmd(nc, in_maps, list(range(N_CORES)), trace=trace,
                               **trace_kwargs)
    out = np.concatenate([res.results[c]["context"] for c in range(N_CORES)],
                         axis=0)
    return out.astype(np.float32), res


def kernel(enc_hs, dec_ht, mask, Wa):
    out, _ = run_on_hw(enc_hs, dec_ht, mask, Wa, trace=False)
    return out


# revision 18
# speedup vs baseline: 1.0082x; 1.0082x over previous
"""Luong 'general' attention kernel for Trainium2 (Bass/Tile), 8-core SPMD.

Math (per batch b):
    v_b        = Wa @ dec_ht[b]                      # (H,)  -- tiny, replaces the
                                                     # huge (T,H)@(H,U) matmul
    raw[t]     = enc_hs[b,t,:] . v_b                 # (T,)
    score[t]   = mask[b,t] ? raw[t] : -1e9           # exact: masked enc gives raw*0
    attn       = softmax(score)                      # masked lanes underflow to 0
    context[b] = sum_t attn[t] * enc_hs[b,t,:]       # masked lanes contribute 0*enc

Sharding: data-parallel over batch B=32 across 8 cores (4 batches/core),
Wa replicated.  Single pass over enc_hs per core (33.5 MB streamed).

Engine plan per core:
  - DVE    : enc*v products for scores (fp32, full precision)
  - ACT    : free-dim reduce of the products (activation Copy + accum), exp
  - PE     : Wa transpose + V matmuls (fp32), context matmuls in f32r
             (single-pass fp32 matmul: 1 cycle/row vs 4 for exact fp32,
             ~2e-4 rel err -- scores still read the same tiles as exact
             f32 via bitcast)
  - DMA    : streams enc (32 MB) + Wa (4 MB) -> the ~100 us roofline
"""

import os
import sys
from contextlib import ExitStack

for _p in ("/root/.axon_site", "/root/.axon_site/_ro/trn_rl_repo",
           "/root/.axon_site/_ro/pypackages", "/opt/trn_rl_repo"):
    if os.path.isdir(_p) and _p not in sys.path:
        sys.path.append(_p)

import numpy as np

import concourse.bass as bass
import concourse.tile as tile
from concourse import bacc, masks, mybir

B, T, H, U = 32, 2048, 1024, 1024
N_CORES = 8
B_LOC = B // N_CORES          # 4 batches per core
TCH = T // 128                # 16 t-chunks of 128 per batch
NEG_BIG = -1.0e9
F32 = mybir.dt.float32
BF16 = mybir.dt.bfloat16
F32R = mybir.dt.float32r


def emit_kernel(tc, enc, dec, mask, wa, out):
    """Emit the per-core program.  enc:[B_LOC,T,H] dec:[B_LOC,H] mask:[B_LOC,T]u8
    wa:[H,U] out:[B_LOC,H], all DRAM APs."""
    nc = tc.nc
    with ExitStack() as ctx:
        const_pool = ctx.enter_context(tc.tile_pool(name="const", bufs=1))
        ident = const_pool.tile([128, 128], F32, tag="ident")
        masks.make_identity(nc, ident[:])
        ones_col = const_pool.tile([128, 1], F32, tag="ones_col")
        nc.vector.memset(ones_col[:], 1.0)
        neg_row = const_pool.tile([1, 128], F32, tag="neg_row")
        nc.vector.memset(neg_row[:], -1.0)
        neg_big = const_pool.tile([128, TCH], F32, tag="neg_big")
        nc.vector.memset(neg_big[:], NEG_BIG)

        # enc tiles are declared float32r so the context matmul takes the
        # fast single-pass fp32 path; the score ops bitcast them to f32.
        # Two pools: the second opens after the V-phase transients release,
        # reusing that SBUF for deeper DMA prefetch.
        enc_pool = ctx.enter_context(tc.tile_pool(name="enc", bufs=20))
        vrep_pool = ctx.enter_context(tc.tile_pool(name="vrep", bufs=1))
        scr_pool = ctx.enter_context(tc.tile_pool(name="scr", bufs=3))
        small_pool = ctx.enter_context(tc.tile_pool(name="small", bufs=2))

        # ---------- Phase V: v_rep[b][p, h] = (Wa @ dec[b])[h] for all p ----------
        vreps = []
        with ExitStack() as vctx:
            wa_pool = vctx.enter_context(tc.tile_pool(name="wa", bufs=1))
            waT_pool = vctx.enter_context(tc.tile_pool(name="waT", bufs=2))
            psum_tr = vctx.enter_context(
                tc.tile_pool(name="psum_tr", bufs=4, space="PSUM"))
            psum_v = vctx.enter_context(
                tc.tile_pool(name="psum_v", bufs=1, space="PSUM"))
            vsb_pool = vctx.enter_context(tc.tile_pool(name="vsb", bufs=1))

            wa_tiles = []
            for i in range(8):  # h-chunk
                wt = wa_pool.tile([128, U], F32, name=f"wa_{i}", tag=f"wa_{i}")
                for hf in range(4):  # split across four DMA queues
                    nc.sync.dma_start(
                        wt[hf * 32:(hf + 1) * 32, :],
                        wa[i * 128 + hf * 32:i * 128 + (hf + 1) * 32, :])
                wa_tiles.append(wt)

            # dec transposed: dT[p, c, b] = dec[b, c*128+p].  Emitted after
            # the Wa loads: these lower to DIRECT2D ops that execute inline
            # on the Sync sequencer and would otherwise delay the bulk
            # transfer triggers.
            dT = vsb_pool.tile([128, 8, B_LOC], F32R, tag="dT")
            for c in range(8):
                nc.sync.dma_start(
                    dT[:, c, :],
                    dec[:, c * 128:(c + 1) * 128].rearrange("b p -> p b"))

            # vT[b, h] accumulated over u-chunks j
            vT_ps = psum_v.tile([B_LOC, H], F32, tag="vT_ps")
            for j in range(8):  # u-chunk
                waT_sb = waT_pool.tile([128, H], F32R, name="waT_sb",
                                       tag="waT_sb", bufs=2)
                for hh in range(2):
                    tr_ps = psum_tr.tile([128, 512], F32, name="tr_ps",
                                         tag="tr_ps", bufs=4)
                    for k in range(4):
                        i = hh * 4 + k  # h-chunk
                        nc.tensor.transpose(
                            tr_ps[:, k * 128:(k + 1) * 128],
                            wa_tiles[i][:, j * 128:(j + 1) * 128],
                            ident[:])
                    # evacuate on DVE / ACT alternately (both idle here)
                    eng = nc.vector if hh == 0 else nc.scalar
                    if eng is nc.vector:
                        nc.vector.tensor_copy(
                            waT_sb[:, hh * 512:(hh + 1) * 512], tr_ps[:])
                    else:
                        nc.scalar.copy(
                            waT_sb[:, hh * 512:(hh + 1) * 512], tr_ps[:])
                for hh in range(2):
                    nc.tensor.matmul(
                        vT_ps[:, hh * 512:(hh + 1) * 512],
                        dT[:, j, :],
                        waT_sb[:, hh * 512:(hh + 1) * 512],
                        start=(j == 0), stop=(j == 7))

            vT_sb = vsb_pool.tile([B_LOC, H], F32, tag="vT_sb")
            nc.scalar.copy(vT_sb[:], vT_ps[:])

            for b in range(B_LOC):
                # row b -> partition 0 (engines can only address partition
                # bases 0/32/64), then broadcast to all 128 partitions
                vb_sb = vsb_pool.tile([1, H], F32, name=f"vb_{b}",
                                      tag=f"vb_{b}")
                nc.sync.dma_start(vb_sb[:], vT_sb[b:b + 1, :])
                vrep = vrep_pool.tile([128, H], F32, name=f"vrep_{b}",
                                      tag=f"vrep_{b}")
                nc.gpsimd.partition_broadcast(vrep[:], vb_sb[:])
                vreps.append(vrep)

        # PSUM pools for the steady state, opened after the V-phase PSUM
        # pools released (only 8 banks exist)
        psum_small = ctx.enter_context(
            tc.tile_pool(name="psum_small", bufs=2, space="PSUM"))
        psum_ctx = ctx.enter_context(
            tc.tile_pool(name="psum_ctx", bufs=2, space="PSUM"))
        enc_pool2 = ctx.enter_context(tc.tile_pool(name="enc2", bufs=21))

        # ---------- Main per-batch pipeline ----------
        for b in range(B_LOC):
            enc_tiles = []
            for i in range(TCH):
                # round-robin tiles across the two pools
                if (b * TCH + i) % 41 < 20:
                    et = enc_pool.tile([128, H], F32R, name="enc_t",
                                       tag="enc_t", bufs=20)
                else:
                    et = enc_pool2.tile([128, H], F32R, name="enc_t2",
                                        tag="enc_t2", bufs=21)
                nc.sync.dma_start(et[:], enc[b, i * 128:(i + 1) * 128, :])
                enc_tiles.append(et)

            # mask (uint8 0/1), laid out [p, i] ~ t = i*128+p
            mk_u8 = small_pool.tile([128, TCH], mybir.dt.uint8, name="mk_u8",
                                    tag="mk_u8", bufs=2)
            nc.sync.dma_start(mk_u8[:],
                               mask[b].rearrange("(i p) -> p i", p=128))

            # scores: DVE multiplies enc*v, ACT reduces over the free dim via
            # activation(Copy) with accum_out (output goes to a broadcast sink)
            sm = small_pool.tile([128, TCH], F32, name="sm", tag="sm", bufs=2)
            for i in range(TCH):
                scr = scr_pool.tile([128, H], F32, name="scr", tag="scr", bufs=3)
                nc.vector.tensor_mul(scr[:], enc_tiles[i][:].bitcast(F32),
                                     vreps[b][:])
                sink = scr_pool.tile([128, 1], F32, name="sink", tag="sink",
                                     bufs=3)
                nc.scalar.activation(sink.broadcast_to(scr.shape), scr[:],
                                     mybir.ActivationFunctionType.Copy,
                                     bias=0.0, scale=1.0,
                                     accum_out=sm[:, i:i + 1])

            # masked score: where(mask, sm, -1e9)
            smm = small_pool.tile([128, TCH], F32, name="smm", tag="smm", bufs=2)
            nc.vector.tensor_copy(smm[:], neg_big[:])
            nc.vector.copy_predicated(smm[:], mk_u8[:], sm[:])

            # global max over [128, TCH]
            colmax = small_pool.tile([128, 1], F32, name="colmax", tag="colmax",
                                     bufs=2)
            nc.vector.reduce_max(colmax[:], smm[:], axis=mybir.AxisListType.X)
            sp = psum_small.tile([128, 160], F32, name="sp", tag="sp", bufs=2)
            nc.tensor.transpose(sp[:1, 0:128], colmax[:], ident[:])
            gmax = small_pool.tile([1, 1], F32, name="gmax", tag="gmax", bufs=2)
            nc.vector.reduce_max(gmax[:], sp[:1, 0:128],
                                 axis=mybir.AxisListType.X)
            # negmax[p] = -gmax, replicated across partitions
            nc.tensor.matmul(sp[:, 128:129], neg_row[:], gmax[:])
            negmax = small_pool.tile([128, 1], F32, name="negmax", tag="negmax",
                                     bufs=2)
            nc.vector.tensor_copy(negmax[:], sp[:, 128:129])

            # p = exp(score - max) in f32r (feeds the f32r context matmul),
            # with the exact fp32 row-sum from the ACT accumulator
            emat = small_pool.tile([128, TCH], F32R, name="emat", tag="emat",
                                   bufs=2)
            rowsum = small_pool.tile([128, 1], F32, name="rowsum", tag="rowsum",
                                     bufs=2)
            nc.scalar.activation(emat[:], smm[:],
                                 mybir.ActivationFunctionType.Exp,
                                 bias=negmax[:], scale=1.0,
                                 accum_out=rowsum[:])

            # denom = sum_p rowsum[p]; rden = 1/denom
            nc.tensor.matmul(sp[:1, 129:130], rowsum[:], ones_col[:])
            rden = small_pool.tile([1, 1], F32, name="rden", tag="rden", bufs=2)
            nc.vector.reciprocal(rden[:], sp[:1, 129:130])

            # context[h] = sum_t p[t] * enc[t, h]  (f32r single-pass matmul)
            cps = psum_ctx.tile([1, H], F32, name="cps", tag="cps", bufs=2)
            for i in range(TCH):
                for hh in range(2):
                    nc.tensor.matmul(
                        cps[:, hh * 512:(hh + 1) * 512],
                        emat[:, i:i + 1],
                        enc_tiles[i][:, hh * 512:(hh + 1) * 512],
                        start=(i == 0), stop=(i == TCH - 1))

            ctx_sb = small_pool.tile([1, H], F32, name="ctx_sb", tag="ctx_sb",
                                     bufs=2)
            nc.vector.tensor_scalar_mul(ctx_sb[:], cps[:], rden[:])
            nc.gpsimd.dma_start(out[b:b + 1, :], ctx_sb[:])


def build_nc():
    """Build and compile the per-core Bass program."""
    nc = bacc.Bacc("TRN2", target_bir_lowering=False, debug=False,
                   enable_asserts=False, num_devices=N_CORES)
    enc_d = nc.dram_tensor("enc_hs", [B_LOC, T, H], F32R,
                           kind="ExternalInput")
    dec_d = nc.dram_tensor("dec_ht", [B_LOC, H], F32R, kind="ExternalInput")
    mask_d = nc.dram_tensor("mask", [B_LOC, T], mybir.dt.uint8,
                            kind="ExternalInput")
    wa_d = nc.dram_tensor("Wa", [H, U], F32, kind="ExternalInput")
    out_d = nc.dram_tensor("context", [B_LOC, H], F32, kind="ExternalOutput")

    with tile.TileContext(nc) as tc:
        emit_kernel(tc, enc_d.ap(), dec_d.ap(), mask_d.ap(), wa_d.ap(),
                    out_d.ap())
    nc.compile()
    return nc


def make_in_maps(enc_hs, dec_ht, mask, Wa):
    """Shard full inputs into per-core input maps (data-parallel over batch)."""
    enc_hs = np.ascontiguousarray(enc_hs, dtype=np.float32)
    dec_ht = np.ascontiguousarray(dec_ht, dtype=np.float32)
    mask_u8 = np.ascontiguousarray(mask).astype(np.uint8)
    Wa = np.ascontiguousarray(Wa, dtype=np.float32)
    in_maps = []
    for c in range(N_CORES):
        sl = slice(c * B_LOC, (c + 1) * B_LOC)
        in_maps.append({
            "enc_hs": enc_hs[sl],
            "dec_ht": dec_ht[sl],
            "mask": mask_u8[sl],
            "Wa": Wa,
        })
    return in_maps


_NC_CACHE = None


def get_nc():
    global _NC_CACHE
    if _NC_CACHE is None:
        _NC_CACHE = build_nc()
    return _NC_CACHE


def run_on_hw(enc_hs, dec_ht, mask, Wa, trace=False, **trace_kwargs):
    from concourse.bass_utils import run_bass_kernel_spmd
    nc = get_nc()
    in_maps = make_in_maps(enc_hs, dec_ht, mask, Wa)
    res = run_bass_kernel_spmd(nc, in_maps, list(range(N_CORES)), trace=trace,
                               **trace_kwargs)
    out = np.concatenate([res.results[c]["context"] for c in range(N_CORES)],
                         axis=0)
    return out.astype(np.float32), res


def kernel(enc_hs, dec_ht, mask, Wa):
    out, _ = run_on_hw(enc_hs, dec_ht, mask, Wa, trace=False)
    return out


# revision 22
# speedup vs baseline: 1.0206x; 1.0123x over previous
"""Luong 'general' attention kernel for Trainium2 (Bass/Tile), 8-core SPMD.

Math (per batch b):
    v_b        = Wa @ dec_ht[b]                      # (H,)  -- tiny, replaces the
                                                     # huge (T,H)@(H,U) matmul
    raw[t]     = enc_hs[b,t,:] . v_b                 # (T,)
    score[t]   = mask[b,t] ? raw[t] : -1e9           # exact: masked enc gives raw*0
    attn       = softmax(score)                      # masked lanes underflow to 0
    context[b] = sum_t attn[t] * enc_hs[b,t,:]       # masked lanes contribute 0*enc

Sharding: data-parallel over batch B=32 across 8 cores (4 batches/core),
Wa replicated.  Single pass over enc_hs per core (33.5 MB streamed).

Engine plan per core:
  - DVE    : enc*v products for scores (fp32, full precision)
  - ACT    : free-dim reduce of the products (activation Copy + accum), exp
  - PE     : Wa transpose + V matmuls (fp32), context matmuls in f32r
             (single-pass fp32 matmul: 1 cycle/row vs 4 for exact fp32,
             ~2e-4 rel err -- scores still read the same tiles as exact
             f32 via bitcast)
  - DMA    : streams enc (32 MB) + Wa (4 MB) -> the ~100 us roofline
"""

import os
import sys
from contextlib import ExitStack

for _p in ("/root/.axon_site", "/root/.axon_site/_ro/trn_rl_repo",
           "/root/.axon_site/_ro/pypackages", "/opt/trn_rl_repo"):
    if os.path.isdir(_p) and _p not in sys.path:
        sys.path.append(_p)

import numpy as np

import concourse.bass as bass
import concourse.tile as tile
from concourse import bacc, masks, mybir

B, T, H, U = 32, 2048, 1024, 1024
N_CORES = 8
B_LOC = B // N_CORES          # 4 batches per core
TCH = T // 128                # 16 t-chunks of 128 per batch
NEG_BIG = -1.0e9
F32 = mybir.dt.float32
BF16 = mybir.dt.bfloat16
F32R = mybir.dt.float32r


def emit_kernel(tc, enc, dec, mask, wa, out):
    """Emit the per-core program.  enc:[B_LOC,T,H] dec:[B_LOC,H] mask:[B_LOC,T]u8
    wa:[H,U] out:[B_LOC,H], all DRAM APs."""
    nc = tc.nc
    with ExitStack() as ctx:
        const_pool = ctx.enter_context(tc.tile_pool(name="const", bufs=1))
        ident = const_pool.tile([128, 128], F32, tag="ident")
        masks.make_identity(nc, ident[:])
        ones_col = const_pool.tile([128, 1], F32, tag="ones_col")
        nc.vector.memset(ones_col[:], 1.0)
        neg_row = const_pool.tile([1, 128], F32, tag="neg_row")
        nc.vector.memset(neg_row[:], -1.0)
        neg_big = const_pool.tile([128, TCH], F32, tag="neg_big")
        nc.vector.memset(neg_big[:], NEG_BIG)

        # enc tiles are declared float32r so the context matmul takes the
        # fast single-pass fp32 path; the score ops bitcast them to f32.
        # Two pools: the second opens after the V-phase transients release,
        # reusing that SBUF for deeper DMA prefetch.
        enc_pool = ctx.enter_context(tc.tile_pool(name="enc", bufs=10))
        vrep_pool = ctx.enter_context(tc.tile_pool(name="vrep", bufs=1))
        scr_pool = ctx.enter_context(tc.tile_pool(name="scr", bufs=3))
        small_pool = ctx.enter_context(tc.tile_pool(name="small", bufs=2))

        # ---------- Phase V: v_rep[b][p, h] = (Wa @ dec[b])[h] for all p ----------
        vreps = []
        with ExitStack() as vctx:
            wa_pool = vctx.enter_context(tc.tile_pool(name="wa", bufs=1))
            waT_pool = vctx.enter_context(tc.tile_pool(name="waT", bufs=2))
            psum_tr = vctx.enter_context(
                tc.tile_pool(name="psum_tr", bufs=4, space="PSUM"))
            psum_v = vctx.enter_context(
                tc.tile_pool(name="psum_v", bufs=1, space="PSUM"))
            vsb_pool = vctx.enter_context(tc.tile_pool(name="vsb", bufs=1))

            wa_tiles = []
            for i in range(8):  # h-chunk
                wt = wa_pool.tile([128, U], F32, name=f"wa_{i}", tag=f"wa_{i}")
                for hf in range(2):  # split across two DMA queues
                    nc.sync.dma_start(
                        wt[hf * 64:(hf + 1) * 64, :],
                        wa[i * 128 + hf * 64:i * 128 + (hf + 1) * 64, :])
                wa_tiles.append(wt)

            # dec transposed: dT[p, c, b] = dec[b, c*128+p].  Emitted after
            # the Wa loads: these lower to DIRECT2D ops that execute inline
            # on the Sync sequencer and would otherwise delay the bulk
            # transfer triggers.
            dT = vsb_pool.tile([128, 8, B_LOC], F32R, tag="dT")
            for c in range(8):
                nc.sync.dma_start(
                    dT[:, c, :],
                    dec[:, c * 128:(c + 1) * 128].rearrange("b p -> p b"))

            # vT[b, h] accumulated over u-chunks j
            vT_ps = psum_v.tile([B_LOC, H], F32, tag="vT_ps")
            for j in range(8):  # u-chunk
                waT_sb = waT_pool.tile([128, H], F32R, name="waT_sb",
                                       tag="waT_sb", bufs=2)
                for hh in range(2):
                    tr_ps = psum_tr.tile([128, 512], F32, name="tr_ps",
                                         tag="tr_ps", bufs=4)
                    for k in range(4):
                        i = hh * 4 + k  # h-chunk
                        nc.tensor.transpose(
                            tr_ps[:, k * 128:(k + 1) * 128],
                            wa_tiles[i][:, j * 128:(j + 1) * 128],
                            ident[:])
                    # evacuate on DVE / ACT alternately (both idle here)
                    eng = nc.vector if hh == 0 else nc.scalar
                    if eng is nc.vector:
                        nc.vector.tensor_copy(
                            waT_sb[:, hh * 512:(hh + 1) * 512], tr_ps[:])
                    else:
                        nc.scalar.copy(
                            waT_sb[:, hh * 512:(hh + 1) * 512], tr_ps[:])
                for hh in range(2):
                    nc.tensor.matmul(
                        vT_ps[:, hh * 512:(hh + 1) * 512],
                        dT[:, j, :],
                        waT_sb[:, hh * 512:(hh + 1) * 512],
                        start=(j == 0), stop=(j == 7))

            vT_sb = vsb_pool.tile([B_LOC, H], F32, tag="vT_sb")
            nc.scalar.copy(vT_sb[:], vT_ps[:])

            for b in range(B_LOC):
                # row b -> partition 0 (engines can only address partition
                # bases 0/32/64), then broadcast to all 128 partitions
                vb_sb = vsb_pool.tile([1, H], F32, name=f"vb_{b}",
                                      tag=f"vb_{b}")
                nc.sync.dma_start(vb_sb[:], vT_sb[b:b + 1, :])
                vrep = vrep_pool.tile([128, H], F32, name=f"vrep_{b}",
                                      tag=f"vrep_{b}")
                nc.gpsimd.partition_broadcast(vrep[:], vb_sb[:])
                vreps.append(vrep)

        # PSUM pools for the steady state, opened after the V-phase PSUM
        # pools released (only 8 banks exist)
        psum_small = ctx.enter_context(
            tc.tile_pool(name="psum_small", bufs=2, space="PSUM"))
        psum_ctx = ctx.enter_context(
            tc.tile_pool(name="psum_ctx", bufs=2, space="PSUM"))
        enc_pool2 = ctx.enter_context(tc.tile_pool(name="enc2", bufs=10))

        # ---------- Main per-batch pipeline ----------
        for b in range(B_LOC):
            # double tiles: one 1 MB DMA per pair of t-chunks (halves the
            # Sync-sequencer arming cost of ~0.64 us per transfer)
            enc_tiles = []
            for i2 in range(TCH // 2):
                if (b * TCH // 2 + i2) % 20 < 10:
                    et2 = enc_pool.tile([128, 2, H], F32R, name="enc_t",
                                        tag="enc_t", bufs=10)
                else:
                    et2 = enc_pool2.tile([128, 2, H], F32R, name="enc_t2",
                                         tag="enc_t2", bufs=10)
                nc.sync.dma_start(
                    et2[:],
                    enc[b, i2 * 256:(i2 + 1) * 256, :].rearrange(
                        "(q p) h -> p q h", p=128))
                enc_tiles.append(et2[:, 0, :])
                enc_tiles.append(et2[:, 1, :])

            # mask (uint8 0/1), laid out [p, i] ~ t = i*128+p
            mk_u8 = small_pool.tile([128, TCH], mybir.dt.uint8, name="mk_u8",
                                    tag="mk_u8", bufs=2)
            nc.sync.dma_start(mk_u8[:],
                               mask[b].rearrange("(i p) -> p i", p=128))

            # scores: DVE multiplies enc*v, ACT reduces over the free dim via
            # activation(Copy) with accum_out (output goes to a broadcast sink)
            sm = small_pool.tile([128, TCH], F32, name="sm", tag="sm", bufs=2)
            for i in range(TCH):
                scr = scr_pool.tile([128, H], F32, name="scr", tag="scr", bufs=3)
                nc.vector.tensor_mul(scr[:], enc_tiles[i][:].bitcast(F32),
                                     vreps[b][:])
                sink = scr_pool.tile([128, 1], F32, name="sink", tag="sink",
                                     bufs=3)
                nc.scalar.activation(sink.broadcast_to(scr.shape), scr[:],
                                     mybir.ActivationFunctionType.Copy,
                                     bias=0.0, scale=1.0,
                                     accum_out=sm[:, i:i + 1])

            # masked score: where(mask, sm, -1e9)
            smm = small_pool.tile([128, TCH], F32, name="smm", tag="smm", bufs=2)
            nc.vector.tensor_copy(smm[:], neg_big[:])
            nc.vector.copy_predicated(smm[:], mk_u8[:], sm[:])

            # global max over [128, TCH]
            colmax = small_pool.tile([128, 1], F32, name="colmax", tag="colmax",
                                     bufs=2)
            nc.vector.reduce_max(colmax[:], smm[:], axis=mybir.AxisListType.X)
            sp = psum_small.tile([128, 160], F32, name="sp", tag="sp", bufs=2)
            nc.tensor.transpose(sp[:1, 0:128], colmax[:], ident[:])
            gmax = small_pool.tile([1, 1], F32, name="gmax", tag="gmax", bufs=2)
            nc.vector.reduce_max(gmax[:], sp[:1, 0:128],
                                 axis=mybir.AxisListType.X)
            # negmax[p] = -gmax, replicated across partitions
            nc.tensor.matmul(sp[:, 128:129], neg_row[:], gmax[:])
            negmax = small_pool.tile([128, 1], F32, name="negmax", tag="negmax",
                                     bufs=2)
            nc.vector.tensor_copy(negmax[:], sp[:, 128:129])

            # p = exp(score - max) in f32r (feeds the f32r context matmul),
            # with the exact fp32 row-sum from the ACT accumulator
            emat = small_pool.tile([128, TCH], F32R, name="emat", tag="emat",
                                   bufs=2)
            rowsum = small_pool.tile([128, 1], F32, name="rowsum", tag="rowsum",
                                     bufs=2)
            nc.scalar.activation(emat[:], smm[:],
                                 mybir.ActivationFunctionType.Exp,
                                 bias=negmax[:], scale=1.0,
                                 accum_out=rowsum[:])

            # denom = sum_p rowsum[p]; rden = 1/denom
            nc.tensor.matmul(sp[:1, 129:130], rowsum[:], ones_col[:])
            rden = small_pool.tile([1, 1], F32, name="rden", tag="rden", bufs=2)
            nc.vector.reciprocal(rden[:], sp[:1, 129:130])

            # context[h] = sum_t p[t] * enc[t, h]  (f32r single-pass matmul)
            cps = psum_ctx.tile([1, H], F32, name="cps", tag="cps", bufs=2)
            for i in range(TCH):
                for hh in range(2):
                    nc.tensor.matmul(
                        cps[:, hh * 512:(hh + 1) * 512],
                        emat[:, i:i + 1],
                        enc_tiles[i][:, hh * 512:(hh + 1) * 512],
                        start=(i == 0), stop=(i == TCH - 1))

            ctx_sb = small_pool.tile([1, H], F32, name="ctx_sb", tag="ctx_sb",
                                     bufs=2)
            nc.vector.tensor_scalar_mul(ctx_sb[:], cps[:], rden[:])
            nc.gpsimd.dma_start(out[b:b + 1, :], ctx_sb[:])


def build_nc():
    """Build and compile the per-core Bass program."""
    nc = bacc.Bacc("TRN2", target_bir_lowering=False, debug=False,
                   enable_asserts=False, num_devices=N_CORES)
    enc_d = nc.dram_tensor("enc_hs", [B_LOC, T, H], F32R,
                           kind="ExternalInput")
    dec_d = nc.dram_tensor("dec_ht", [B_LOC, H], F32R, kind="ExternalInput")
    mask_d = nc.dram_tensor("mask", [B_LOC, T], mybir.dt.uint8,
                            kind="ExternalInput")
    wa_d = nc.dram_tensor("Wa", [H, U], F32, kind="ExternalInput")
    out_d = nc.dram_tensor("context", [B_LOC, H], F32, kind="ExternalOutput")

    with tile.TileContext(nc) as tc:
        emit_kernel(tc, enc_d.ap(), dec_d.ap(), mask_d.ap(), wa_d.ap(),
                    out_d.ap())
    nc.compile()
    return nc


def make_in_maps(enc_hs, dec_ht, mask, Wa):
    """Shard full inputs into per-core input maps (data-parallel over batch)."""
    enc_hs = np.ascontiguousarray(enc_hs, dtype=np.float32)
    dec_ht = np.ascontiguousarray(dec_ht, dtype=np.float32)
    mask_u8 = np.ascontiguousarray(mask).astype(np.uint8)
    Wa = np.ascontiguousarray(Wa, dtype=np.float32)
    in_maps = []
    for c in range(N_CORES):
        sl = slice(c * B_LOC, (c + 1) * B_LOC)
        in_maps.append({
            "enc_hs": enc_hs[sl],
            "dec_ht": dec_ht[sl],
            "mask": mask_u8[sl],
            "Wa": Wa,
        })
    return in_maps


_NC_CACHE = None


def get_nc():
    global _NC_CACHE
    if _NC_CACHE is None:
        _NC_CACHE = build_nc()
    return _NC_CACHE


def run_on_hw(enc_hs, dec_ht, mask, Wa, trace=False, **trace_kwargs):
    from concourse.bass_utils import run_bass_kernel_spmd
    nc = get_nc()
    in_maps = make_in_maps(enc_hs, dec_ht, mask, Wa)
    res = run_bass_kernel_spmd(nc, in_maps, list(range(N_CORES)), trace=trace,
                               **trace_kwargs)
    out = np.concatenate([res.results[c]["context"] for c in range(N_CORES)],
                         axis=0)
    return out.astype(np.float32), res


def kernel(enc_hs, dec_ht, mask, Wa):
    out, _ = run_on_hw(enc_hs, dec_ht, mask, Wa, trace=False)
    return out
